# revision 14
# baseline (speedup 1.0000x reference)
"""DPConformer Trainium2 kernel.

Data-parallel over the B*F=256 effective batch axis: each of the 8 cores
processes 32 sequences of (T=250, C=256). All params replicated.

Layouts on device (per sequence):
  - residual stream y: token-major fp32, two tiles [125, 256]
  - matmul operands: bf16; projections consume the transposed (feature-major)
    LN output yT [128c, 250t] as lhsT
  - relative-position bias via Dt = q @ peT (token-major [t, 500]) -> DRAM
    -> diagonal strided re-read (skew), reversal absorbed into a reversed
    free-dim operand of the score add
  - depthwise conv: 33 accumulating diagonal matmuls, channel-major
"""

import math
from contextlib import ExitStack

import numpy as np

import concourse.bass as bass
import concourse.tile as tile
from concourse import bacc, mybir
from concourse.bass_utils import run_bass_kernel_spmd

F32 = mybir.dt.float32
BF16 = mybir.dt.bfloat16
AF = mybir.ActivationFunctionType
OP = mybir.AluOpType

N_HEAD = 4
KER = 33
MAXLEN = 2000
C = 256
DFF = 1024
T = 250
TT = 125  # token tile size (2 tiles per sequence)
DK = C // N_HEAD  # 64
NCORES = 8
SEQS = 32  # per core
LN_EPS = 1e-5
BN_EPS = 1e-5
PAD = (KER - 1) // 2  # 16
TPAD = T + 2 * PAD  # 282
NR = 500  # relative-position table width (499 used, padded to 500)
NRW = 375  # per-token-tile Dt width: r' = t_local + s' with t_local<125, s'<250


def _np(a):
    return np.asarray(a, dtype=np.float32)


def _prep_params(params):
    """Host-side param folding. Returns dict of np arrays (device inputs)."""
    out = {}

    def fold_ln(g, b, w, bias):
        # LN(x)*g+b then @w+bias  ==  xhat @ (g[:,None]*w) + (b@w + bias)
        return (g[:, None] * w).astype(np.float32), (b @ w + bias).astype(np.float32)

    def colmajor_bias(b, ntile):
        # per-partition bias columns: b[cout] -> [128, ntile]
        return np.ascontiguousarray(b.reshape(ntile, 128).T).astype(np.float32)

    for name in ("ffn_in", "ffn_out"):
        p = params[name]
        g, b = _np(p["ln_g"]), _np(p["ln_b"])
        w1, b1 = fold_ln(g, b, _np(p["w1"]), _np(p["b1"]))
        sfx = "i" if name == "ffn_in" else "o"
        out[f"w1{sfx}"] = w1
        out[f"b1{sfx}"] = colmajor_bias(b1, DFF // 128)
        out[f"w2{sfx}"] = _np(p["w2"])
        out[f"b2{sfx}"] = _np(p["b2"])[None, :]

    p = params["attn"]
    g, b = _np(p["ln_g"]), _np(p["ln_b"])
    scale = 1.0 / math.sqrt(DK)
    wq, bq = fold_ln(g, b, _np(p["wq"]), _np(p["bq"]))
    out["wq"] = wq * scale
    out["bq"] = colmajor_bias(bq * scale, 2)
    wk, bk = fold_ln(g, b, _np(p["wk"]), _np(p["bk"]))
    out["wk"] = wk
    out["bk"] = colmajor_bias(bk, 2)
    wv, bv = fold_ln(g, b, _np(p["wv"]), _np(p["bv"]))
    out["wv"] = wv
    # attn rows sum to 1 -> v bias is additive after AV; fold into wo bias
    out["wo"] = _np(p["wo"])
    out["bo"] = (_np(p["bo"]) + bv @ _np(p["wo"]))[None, :]

    # pe table: peT[d, r] = pe_k[1751 + r, d], duplicated on both partition halves
    pe = _np(params["pe_k"])  # (4000, 64)
    pe_used = np.zeros((NR, DK), np.float32)
    pe_used[:499] = pe[1751:2250]
    peT = np.ascontiguousarray(pe_used.T)  # (64, 500)
    out["peT2"] = np.concatenate([peT, peT], axis=0)  # (128, 500)

    p = params["conv"]
    g, b = _np(p["ln_g"]), _np(p["ln_b"])
    pw1, pb1 = fold_ln(g, b, _np(p["pw1_w"]), _np(p["pw1_b"]))
    out["pw1"] = pw1
    out["pb1"] = pb1[None, :]
    dw = _np(p["dw_w"])[:, 0, :]  # (C, 33)
    s = _np(p["bn_g"]) / np.sqrt(_np(p["bn_var"]) + BN_EPS)
    dws = dw * s[:, None]
    bconv = (_np(p["dw_b"]) - _np(p["bn_mean"])) * s + _np(p["bn_b"])
    out["bconv"] = colmajor_bias(bconv, 2)
    # packed depthwise weights: 8 tap-groups of 4 taps x 32-channel groups.
    # W4[ct, tg, g, i*32+a, a] = w'[ct*128+g*32+a, 4*tg+i]; tap 32 is a
    # separate [128,128] diagonal per ct.
    w4 = np.zeros((2, 8, 4, 128, 32), np.float32)
    for ct in range(2):
        for tg in range(8):
            for g in range(4):
                for i in range(4):
                    ch = ct * 128 + g * 32 + np.arange(32)
                    w4[ct, tg, g, i * 32 + np.arange(32), np.arange(32)] = dws[
                        ch, 4 * tg + i
                    ]
    out["dwpack"] = w4
    diag = np.zeros((2, 128, 128), np.float32)
    idx = np.arange(128)
    for ct in range(2):
        diag[ct, idx, idx] = dws[ct * 128 : (ct + 1) * 128, KER - 1]
    out["dwdiag"] = diag
    out["pw2"] = _np(p["pw2_w"])
    out["pb2"] = _np(p["pw2_b"])[None, :]

    out["lng"] = _np(params["ln_g"])[None, :]
    out["lnb"] = _np(params["ln_b"])[None, :]
    out["ident"] = np.eye(128, dtype=np.float32)
    out["ones"] = np.ones((1, 128), np.float32)
    return out


# name -> (shape, is_bf16): device-side constant tensors
def _const_specs():
    sp = {
        "w1i": ((C, DFF), True), "b1i": ((128, 8), False),
        "w2i": ((DFF, C), True), "b2i": ((1, C), True),
        "w1o": ((C, DFF), True), "b1o": ((128, 8), False),
        "w2o": ((DFF, C), True), "b2o": ((1, C), True),
        "wq": ((C, C), True), "bq": ((128, 2), False),
        "wk": ((C, C), True), "bk": ((128, 2), False),
        "wv": ((C, C), True),
        "wo": ((C, C), True), "bo": ((1, C), True),
        "peT2": ((128, NR), True),
        "pw1": ((C, 2 * C), True), "pb1": ((1, 2 * C), True),
        "dwpack": ((2, 8, 4, 128, 32), True),
        "dwdiag": ((2, 128, 128), True), "bconv": ((128, 2), False),
        "pw2": ((C, C), True), "pb2": ((1, C), True),
        "lng": ((1, C), False), "lnb": ((1, C), False),
        "ident": ((128, 128), True), "ones": ((1, 128), True),
    }
    return sp


def build_program(n_seqs=SEQS, use_mask=False):
    nc = bacc.Bacc("TRN2", target_bir_lowering=False, debug=False)

    xs = nc.dram_tensor("xs", [n_seqs, T, C], F32, kind="ExternalInput")
    ys = nc.dram_tensor("ys", [n_seqs, T, C], F32, kind="ExternalOutput")
    consts_dram = {}
    for name, (shape, is_bf) in _const_specs().items():
        consts_dram[name] = nc.dram_tensor(
            name, list(shape), BF16 if is_bf else F32, kind="ExternalInput"
        )
    if use_mask:
        mb_dram = nc.dram_tensor("maskb", [n_seqs, T, T], BF16, kind="ExternalInput")

    with tile.TileContext(nc) as tc, ExitStack() as ctx:
        cp = ctx.enter_context(tc.tile_pool(name="consts", bufs=1))
        # ---- load all constants into SBUF ----
        cs = {}
        for name, (shape, is_bf) in _const_specs().items():
            dt = BF16 if is_bf else F32
            d = consts_dram[name]
            if name in ("w1i", "w1o", "wq", "wk", "wv", "wo", "pw1", "pw2"):
                # [cin, cout] -> 2 K-tiles [128, cout]
                cs[name] = [cp.tile([128, shape[1]], dt, tag=f"{name}{k}", name=f"{name}{k}") for k in range(2)]
                for k in range(2):
                    nc.sync.dma_start(out=cs[name][k], in_=d.ap()[k * 128 : (k + 1) * 128, :])
            elif name in ("w2i", "w2o"):
                cs[name] = [cp.tile([128, C], dt, tag=f"{name}{k}", name=f"{name}{k}") for k in range(8)]
                for k in range(8):
                    nc.sync.dma_start(out=cs[name][k], in_=d.ap()[k * 128 : (k + 1) * 128, :])
            elif name == "dwpack":
                cs[name] = [
                    [
                        [
                            cp.tile([128, 32], dt, tag=f"w4_{ct}_{tg}_{g}", name=f"w4_{ct}_{tg}_{g}")
                            for g in range(4)
                        ]
                        for tg in range(8)
                    ]
                    for ct in range(2)
                ]
                for ct in range(2):
                    for tg in range(8):
                        for g in range(4):
                            nc.sync.dma_start(
                                out=cs[name][ct][tg][g], in_=d.ap()[ct, tg, g, :, :]
                            )
            elif name == "dwdiag":
                cs[name] = [
                    cp.tile([128, 128], dt, tag=f"dwd{ct}", name=f"dwd{ct}") for ct in range(2)
                ]
                for ct in range(2):
                    nc.sync.dma_start(out=cs[name][ct], in_=d.ap()[ct, :, :])
            elif name in ("lng", "lnb"):
                tl = cp.tile([128, C], dt, tag=name)
                bc = bass.AP(tensor=d.ap().tensor, offset=0, ap=[[0, 128], [1, C]])
                nc.sync.dma_start(out=tl, in_=bc)
                cs[name] = tl
            else:
                tl = cp.tile(list(shape), dt, tag=name)
                nc.sync.dma_start(out=tl, in_=d.ap())
                cs[name] = tl

        eps_t = cp.tile([128, 1], F32, tag="eps")
        nc.vector.memset(eps_t, LN_EPS)

        ident = cs["ident"]
        ones = cs["ones"]

        # ---- pools ----
        yp = ctx.enter_context(tc.tile_pool(name="y", bufs=4))
        wk_sb = ctx.enter_context(tc.tile_pool(name="work", bufs=3))
        ht_p = ctx.enter_context(tc.tile_pool(name="ht", bufs=10))
        at_p = ctx.enter_context(tc.tile_pool(name="attn", bufs=4))
        sm_p = ctx.enter_context(tc.tile_pool(name="small", bufs=6))
        ps_tr = ctx.enter_context(tc.tile_pool(name="ps_tr", bufs=2, space=bass.MemorySpace.PSUM))
        ps_mm = ctx.enter_context(tc.tile_pool(name="ps_mm", bufs=3, space=bass.MemorySpace.PSUM))
        ps_dt = ctx.enter_context(tc.tile_pool(name="ps_dt", bufs=2, space=bass.MemorySpace.PSUM))
        ps_av = ctx.enter_context(tc.tile_pool(name="ps_av", bufs=1, space=bass.MemorySpace.PSUM))
        dr_p = ctx.enter_context(tc.tile_pool(name="dram", bufs=8, space=bass.MemorySpace.DRAM))

        def rev_free(ap):
            """Reverse the innermost (free) dim of a 2-D AP."""
            (p0, pc), (f0, fc) = ap.ap[0], ap.ap[1]
            assert f0 == 1
            return bass.AP(
                tensor=ap.tensor, offset=ap.offset + fc - 1, ap=[[p0, pc], [-1, fc]]
            )

        def layer_norm_t(x_t, out_dt=BF16):
            """token-major LN on [TT, 256] fp32 -> normed tile (no g/b)."""
            st = sm_p.tile([128, 6], F32, tag="st")
            nc.vector.bn_stats(out=st[:TT, :], in_=x_t[:TT, :])
            mv = sm_p.tile([128, 2], F32, tag="mv")
            nc.vector.bn_aggr(out=mv[:TT, :], in_=st[:TT, :])
            sd = sm_p.tile([128, 1], F32, tag="sd")
            nc.scalar.activation(
                out=sd[:TT], in_=mv[:TT, 1:2], func=AF.Sqrt, bias=eps_t[:TT], scale=1.0
            )
            rs = sm_p.tile([128, 1], F32, tag="rs")
            nc.vector.reciprocal(out=rs[:TT], in_=sd[:TT])
            nm = wk_sb.tile([128, C], out_dt, tag="normed")
            nc.vector.tensor_scalar(
                out=nm[:TT, :], in0=x_t[:TT, :], scalar1=mv[:TT, 0:1], scalar2=rs[:TT],
                op0=OP.subtract, op1=OP.mult,
            )
            return nm

        def transpose_to(yT, nm, tt):
            """nm [TT, 256] bf16 -> write columns tt*TT of yT tiles [128, 250]."""
            for kt in range(2):
                ps = ps_tr.tile([128, 128], BF16, tag="tr")
                nc.tensor.transpose(
                    ps[:128, :TT], nm[:TT, kt * 128 : (kt + 1) * 128], ident[:TT, :TT]
                )
                nc.vector.tensor_copy(
                    out=yT[kt][:, tt * TT : (tt + 1) * TT], in_=ps[:128, :TT]
                )

        def ln_transpose(y, tag):
            """LN both token tiles and build feature-major yT [128, 250] x2."""
            yT = [wk_sb.tile([128, T], BF16, tag=f"{tag}{k}", name=f"{tag}{k}") for k in range(2)]
            for tt in range(2):
                nm = layer_norm_t(y[tt])
                transpose_to(yT, nm, tt)
            return yT

        def ffn(y, w1, b1, w2, b2row, resid_scale):
            yT = ln_transpose(y, "yTf")
            h = []
            for mt in range(8):
                ps = ps_mm.tile([128, 512], F32, tag="mm")
                for kt in range(2):
                    nc.tensor.matmul(
                        ps[:128, :T],
                        w1[kt][:, mt * 128 : (mt + 1) * 128],
                        yT[kt],
                        start=(kt == 0),
                        stop=(kt == 1),
                    )
                ht = ht_p.tile([128, T], BF16, tag="h")
                nc.vector.tensor_scalar(
                    out=ht, in0=ps[:128, :T], scalar1=b1[:, mt : mt + 1], scalar2=0.0,
                    op0=OP.add, op1=OP.max,
                )
                h.append(ht)
            for tt in range(2):
                ps = ps_mm.tile([128, 512], F32, tag="mm")
                for kt in range(8):
                    nc.tensor.matmul(
                        ps[:TT, :C],
                        h[kt][:, tt * TT : (tt + 1) * TT],
                        w2[kt],
                        start=(kt == 0),
                        stop=False,
                    )
                nc.tensor.matmul(
                    ps[:TT, :C], ones[0:1, :TT], b2row, start=False, stop=True
                )
                nc.vector.scalar_tensor_tensor(
                    out=y[tt][:TT, :], in0=ps[:TT, :C], scalar=resid_scale,
                    in1=y[tt][:TT, :], op0=OP.mult, op1=OP.add,
                )

        def attention(y, seq):
            yT = ln_transpose(y, "yTa")
            # projections
            qT, kT = [], []
            for pt in range(2):
                for dst, w, b in ((qT, cs["wq"], cs["bq"]), (kT, cs["wk"], cs["bk"])):
                    ps = ps_mm.tile([128, 512], F32, tag="mm")
                    for kt in range(2):
                        nc.tensor.matmul(
                            ps[:128, :T],
                            w[kt][:, pt * 128 : (pt + 1) * 128],
                            yT[kt],
                            start=(kt == 0),
                            stop=(kt == 1),
                        )
                    sb = at_p.tile([128, T], BF16, tag="qk")
                    nc.vector.tensor_scalar(
                        out=sb, in0=ps[:128, :T], scalar1=b[:, pt : pt + 1],
                        scalar2=None, op0=OP.add,
                    )
                    dst.append(sb)
            v = []
            for tt in range(2):
                ps = ps_mm.tile([128, 512], F32, tag="mm")
                for kt in range(2):
                    nc.tensor.matmul(
                        ps[:TT, :C],
                        yT[kt][:, tt * TT : (tt + 1) * TT],
                        cs["wv"][kt],
                        start=(kt == 0),
                        stop=(kt == 1),
                    )
                sb = at_p.tile([128, C], BF16, tag="v")
                nc.vector.tensor_copy(out=sb[:TT, :], in_=ps[:TT, :C])
                v.append(sb)

            if use_mask:
                mbt = []
                for tt in range(2):
                    mt_ = at_p.tile([128, T], BF16, tag="mb")
                    nc.sync.dma_start(
                        out=mt_[:TT, :], in_=mb_dram.ap()[seq, tt * TT : (tt + 1) * TT, :]
                    )
                    mbt.append(mt_)

            av_sb = []
            for pt in range(2):
                avps = ps_av.tile([128, T], F32, tag="av")
                for hh in range(2):
                    h = pt * 2 + hh
                    hb = hh * 64
                    for tt in range(2):
                        tsl = slice(tt * TT, (tt + 1) * TT)
                        # Dt = q @ peT slice  (token-major [t_local, 375]):
                        # Dt[tl, r] = q_t . pe_used[tt*125 + tl + ... shifted]
                        dps = ps_dt.tile([128, NRW], F32, tag="dt")
                        nc.tensor.matmul(
                            dps[:TT, :NRW],
                            qT[pt][hb : hb + 64, tsl],
                            cs["peT2"][hb : hb + 64, tt * TT : tt * TT + NRW],
                        )
                        dsb = wk_sb.tile([128, NRW], BF16, tag="dsb")
                        nc.vector.tensor_copy(out=dsb[:TT, :], in_=dps[:TT, :NRW])
                        dt_d = dr_p.tile([TT, NRW], BF16, tag="dtd")
                        nc.sync.dma_start(out=dt_d, in_=dsb[:TT, :])
                        # skew read: REV_Bt[tl, s'] = Dt[tl, tl + s'] = flat[376*tl + s']
                        rb = wk_sb.tile([128, T], BF16, tag="rb")
                        skew = bass.AP(
                            tensor=dt_d.tensor, offset=dt_d.offset,
                            ap=[[NRW + 1, TT], [1, T]],
                        )
                        nc.sync.dma_start(out=rb[:TT, :], in_=skew)
                        # A = q @ k^T
                        aps = ps_mm.tile([128, 512], F32, tag="mm")
                        nc.tensor.matmul(
                            aps[:TT, :T], qT[pt][hb : hb + 64, tsl], kT[pt][hb : hb + 64, :]
                        )
                        # scores = A + reverse(REV_Bt)
                        sc = wk_sb.tile([128, T], F32, tag="sc")
                        nc.vector.scalar_tensor_tensor(
                            out=sc[:TT, :], in0=aps[:TT, :T], scalar=0.0,
                            in1=rev_free(rb[:TT, :]), op0=OP.add, op1=OP.add,
                        )
                        if use_mask:
                            nc.vector.tensor_tensor(
                                out=sc[:TT, :], in0=sc[:TT, :], in1=mbt[tt][:TT, :],
                                op=OP.add,
                            )
                        # softmax (no max subtraction: |scores| is small)
                        ex = wk_sb.tile([128, T], BF16, tag="ex")
                        rsum = sm_p.tile([128, 1], F32, tag="rsum")
                        nc.scalar.activation(
                            out=ex[:TT, :], in_=sc[:TT, :], func=AF.Exp,
                            accum_out=rsum[:TT],
                        )
                        rcp = sm_p.tile([128, 1], F32, tag="rcp")
                        if use_mask:
                            nc.vector.tensor_scalar(
                                out=rsum[:TT], in0=rsum[:TT], scalar1=1e-30,
                                scalar2=None, op0=OP.add,
                            )
                        nc.vector.reciprocal(out=rcp[:TT], in_=rsum[:TT])
                        an = wk_sb.tile([128, T], BF16, tag="an")
                        nc.vector.tensor_scalar(
                            out=an[:TT, :], in0=ex[:TT, :], scalar1=rcp[:TT],
                            scalar2=None, op0=OP.mult,
                        )
                        # attn^T tiles and AV accumulation
                        for ks in range(2):
                            tps = ps_tr.tile([128, 128], BF16, tag="tr")
                            nc.tensor.transpose(
                                tps[:TT, :TT], an[:TT, ks * TT : (ks + 1) * TT],
                                ident[:TT, :TT],
                            )
                            ats = wk_sb.tile([128, TT], BF16, tag="ats")
                            nc.vector.tensor_copy(out=ats[:TT, :], in_=tps[:TT, :TT])
                            nc.tensor.matmul(
                                avps[hb : hb + 64, tsl],
                                v[ks][:TT, h * 64 : (h + 1) * 64],
                                ats[:TT, :],
                                start=(ks == 0),
                                stop=(ks == 1),
                            )
                sb = at_p.tile([128, T], BF16, tag="avsb")
                nc.vector.tensor_copy(out=sb, in_=avps[:128, :T])
                av_sb.append(sb)
            # out projection + residual
            for tt in range(2):
                ps = ps_mm.tile([128, 512], F32, tag="mm")
                for pt in range(2):
                    nc.tensor.matmul(
                        ps[:TT, :C],
                        av_sb[pt][:, tt * TT : (tt + 1) * TT],
                        cs["wo"][pt],
                        start=(pt == 0),
                        stop=False,
                    )
                nc.tensor.matmul(
                    ps[:TT, :C], ones[0:1, :TT], cs["bo"], start=False, stop=True
                )
                nc.vector.scalar_tensor_tensor(
                    out=y[tt][:TT, :], in0=ps[:TT, :C], scalar=1.0,
                    in1=y[tt][:TT, :], op0=OP.mult, op1=OP.add,
                )

        def conv_block(y):
            yT = ln_transpose(y, "yTc")
            # pointwise 1 (token-major) + GLU
            glupad = [wk_sb.tile([128, TPAD], BF16, tag=f"gp{ct}", name=f"gp{ct}") for ct in range(2)]
            for ct in range(2):
                nc.vector.memset(glupad[ct][:, 0:PAD], 0.0)
                nc.vector.memset(glupad[ct][:, PAD + T : TPAD], 0.0)
            for tt in range(2):
                ps = ps_mm.tile([128, 512], F32, tag="mm")
                for kt in range(2):
                    nc.tensor.matmul(
                        ps[:TT, : 2 * C],
                        yT[kt][:, tt * TT : (tt + 1) * TT],
                        cs["pw1"][kt],
                        start=(kt == 0),
                        stop=False,
                    )
                nc.tensor.matmul(
                    ps[:TT, : 2 * C], ones[0:1, :TT], cs["pb1"], start=False, stop=True
                )
                sg = wk_sb.tile([128, C], BF16, tag="sg")
                nc.scalar.activation(out=sg[:TT, :], in_=ps[:TT, 0:C], func=AF.Sigmoid)
                gl = wk_sb.tile([128, C], BF16, tag="gl")
                nc.vector.tensor_tensor(
                    out=gl[:TT, :], in0=ps[:TT, C : 2 * C], in1=sg[:TT, :], op=OP.mult
                )
                # transpose GLU out into padded channel-major tiles
                for kt in range(2):
                    tps = ps_tr.tile([128, 128], BF16, tag="tr")
                    nc.tensor.transpose(
                        tps[:128, :TT], gl[:TT, kt * 128 : (kt + 1) * 128], ident[:TT, :TT]
                    )
                    nc.vector.tensor_copy(
                        out=glupad[kt][:, PAD + tt * TT : PAD + (tt + 1) * TT],
                        in_=tps[:128, :TT],
                    )
            # depthwise conv: taps 0..31 via 4-tap-packed matmuls (rows =
            # 32 channels x 4 pre-shifted copies, col-tiled over 4 channel
            # groups), tap 32 via one diagonal matmul.
            SHW = TPAD - 3  # 279: widest shifted window needed
            convT = []
            for ct in range(2):
                sh4 = [
                    wk_sb.tile([128, SHW], BF16, tag=f"sh4_{ct}_{g}", name=f"sh4_{ct}_{g}")
                    for g in range(4)
                ]
                for g in range(4):
                    for i in range(4):
                        nc.sync.dma_start(
                            out=sh4[g][i * 32 : (i + 1) * 32, :],
                            in_=glupad[ct][g * 32 : (g + 1) * 32, i : i + SHW],
                        )
                ps = ps_mm.tile([128, 512], F32, tag="mm")
                for tg in range(8):
                    for g in range(4):
                        nc.tensor.matmul(
                            ps[g * 32 : (g + 1) * 32, :T],
                            cs["dwpack"][ct][tg][g],
                            sh4[g][:, 4 * tg : 4 * tg + T],
                            start=(tg == 0),
                            stop=False,
                            tile_position=(0, g * 32),
                        )
                nc.tensor.matmul(
                    ps[:128, :T],
                    cs["dwdiag"][ct],
                    glupad[ct][:, KER - 1 : KER - 1 + T],
                    start=False,
                    stop=True,
                )
                cv = wk_sb.tile([128, T], BF16, tag="cv")
                nc.vector.tensor_scalar(
                    out=cv, in0=ps[:128, :T], scalar1=cs["bconv"][:, ct : ct + 1],
                    scalar2=0.0, op0=OP.add, op1=OP.max,
                )
                convT.append(cv)
            # pointwise 2 + residual
            for tt in range(2):
                ps = ps_mm.tile([128, 512], F32, tag="mm")
                for kt in range(2):
                    nc.tensor.matmul(
                        ps[:TT, :C],
                        convT[kt][:, tt * TT : (tt + 1) * TT],
                        cs["pw2"][kt],
                        start=(kt == 0),
                        stop=False,
                    )
                nc.tensor.matmul(
                    ps[:TT, :C], ones[0:1, :TT], cs["pb2"], start=False, stop=True
                )
                nc.vector.scalar_tensor_tensor(
                    out=y[tt][:TT, :], in0=ps[:TT, :C], scalar=1.0,
                    in1=y[tt][:TT, :], op0=OP.mult, op1=OP.add,
                )

        # ================= main sequence loop =================
        for seq in range(n_seqs):
            y = [yp.tile([128, C], F32, tag="y", name="y") for _ in range(2)]
            for tt in range(2):
                nc.sync.dma_start(
                    out=y[tt][:TT, :], in_=xs.ap()[seq, tt * TT : (tt + 1) * TT, :]
                )
            ffn(y, cs["w1i"], cs["b1i"], cs["w2i"], cs["b2i"], 0.5)
            attention(y, seq)
            conv_block(y)
            ffn(y, cs["w1o"], cs["b1o"], cs["w2o"], cs["b2o"], 0.5)
            # final LN with g/b
            for tt in range(2):
                nm = layer_norm_t(y[tt], out_dt=F32)
                o = wk_sb.tile([128, C], F32, tag="out")
                nc.vector.scalar_tensor_tensor(
                    out=o[:TT, :], in0=nm[:TT, :], scalar=1.0, in1=cs["lng"][:TT, :],
                    op0=OP.mult, op1=OP.mult,
                )
                nc.vector.tensor_tensor(
                    out=o[:TT, :], in0=o[:TT, :], in1=cs["lnb"][:TT, :], op=OP.add
                )
                nc.sync.dma_start(
                    out=ys.ap()[seq, tt * TT : (tt + 1) * TT, :], in_=o[:TT, :]
                )

    nc.compile()
    return nc


_PROGRAM_CACHE = {}


def _get_program(n_seqs, use_mask):
    key = (n_seqs, use_mask)
    if key not in _PROGRAM_CACHE:
        _PROGRAM_CACHE[key] = build_program(n_seqs, use_mask)
    return _PROGRAM_CACHE[key]


def _to_bf16_bits(a):
    import ml_dtypes

    return np.asarray(a, dtype=np.float32).astype(ml_dtypes.bfloat16)


def make_in_maps(x, mask, params, n_cores=NCORES, trace=False):
    """Host prep: returns (in_maps, use_mask)."""
    x = np.asarray(x, dtype=np.float32)
    mask = np.asarray(mask)
    B, T_, C_, F_ = x.shape
    seqs = np.ascontiguousarray(x.transpose(0, 3, 1, 2).reshape(B * F_, T_, C_))
    use_mask = not bool((mask == 1).all())

    pp = _prep_params(params)
    const_np = {}
    for name, (shape, is_bf) in _const_specs().items():
        a = pp[name].reshape(shape)
        const_np[name] = _to_bf16_bits(a) if is_bf else np.ascontiguousarray(a, dtype=np.float32)

    per = seqs.shape[0] // n_cores
    in_maps = []
    for c in range(n_cores):
        m = dict(const_np)
        m["xs"] = np.ascontiguousarray(seqs[c * per : (c + 1) * per])
        if use_mask:
            mb = (np.asarray(mask[c * per : (c + 1) * per]) == 0) * np.float32(-1e9)
            m["maskb"] = _to_bf16_bits(mb)
        in_maps.append(m)
    return in_maps, use_mask


def kernel(x, mask, params):
    in_maps, use_mask = make_in_maps(x, mask, params)
    nc = _get_program(SEQS, use_mask)
    res = run_bass_kernel_spmd(nc, in_maps, core_ids=list(range(NCORES)))
    outs = [r["ys"] for r in res.results]
    B, T_, C_, F_ = x.shape
    y = np.concatenate(outs, axis=0)  # (B*F, T, C)
    y = y.reshape(B, F_, T_, C_).transpose(0, 2, 3, 1)
    return np.ascontiguousarray(y.astype(np.float32))
